# revision 31
# baseline (speedup 1.0000x reference)
"""Trainium2 Bass kernel for the per-node compressor + SE-gate + classifier model.

Strategy: data-parallel over batch B across 8 NeuronCores (512 rows each).
On-chip layout is feature-major [feature, batch]; BatchNorm is folded into the
linear weights on the host. The SE pooling means ride for free in spare lhsT
columns of the classifier matmuls: the local classifier emits 118 rows
(100 classes + 18 x-mean SE rows), and one "representative" others-classifier
group per source node emits 118 rows (100 classes + 18 comp-mean SE rows),
so no dedicated mean matmuls are needed. SE gates are produced as
partition-broadcast [100, 512] tiles from tiny matmuls; the classifier is
computed as per-block ungated partials which are gated post-matmul on
VectorE/Pool. Matmul operands are bf16; accumulation and gating stay fp32.
"""

import numpy as np

import concourse.bass as bass
import concourse.tile as tile
from concourse import bacc, mybir
from concourse.bass_utils import run_bass_kernel_spmd

# Problem shapes (hardcoded per harness contract)
B, N, F, FO, C = 4096, 6, 1024, 512, 100
HID = (F + FO) // 2          # 768
RED = N // 2                 # 3
EPS = 1e-5
IDX = np.array([[j for j in range(N) if j != i] for i in range(N)])

NCORES = 8
BL = B // NCORES             # 512 batch rows per core
P = 128
KF = F // P                  # 8 f-chunks
MH = HID // P                # 6 h-chunks
KH = HID // P                # 6
MO = FO // P                 # 4 o-chunks
CE = P                       # classifier psum rows: 100 classes + 18 SE-mean
                             # rows at 100:118 + zero padding to 128 (so the
                             # 32-aligned [96:128] slice is fully defined)

f32 = mybir.dt.float32
bf16 = mybir.dt.bfloat16
MMDT = bf16
AF = mybir.ActivationFunctionType

# representative consumer for source node j: its (n0, j) classifier group is
# computed in stage A (as soon as comp[j] exists) and carries the comp-mean
# SE rows; node n0 then skips that pair in stage C.
N0 = [(j + 1) % N for j in range(N)]

LAST_EXEC_TIME_NS = None

_BUILT = {}


def _build_nc(reps=1, hw_loop=False, cut=None, bodies_per_iter=1):
    # cut: None=full body, "l1"=L1 only, "l1l2"=compressors only,
    # "stagea"=through SE pre-acts (perf bisection only; kernel() uses full)
    nc = bacc.Bacc("TRN2", target_bir_lowering=False, debug=False,
                   num_devices=NCORES)

    xT_d = nc.dram_tensor("xT", [N, P, KF, BL], MMDT, kind="ExternalInput").ap()
    w1_d = nc.dram_tensor("w1", [N, P, MH, KF, P], MMDT, kind="ExternalInput").ap()
    w2_d = nc.dram_tensor("w2", [N, P, MO, KH, P], MMDT, kind="ExternalInput").ap()
    wcl_d = nc.dram_tensor("wcl", [N, P, KF, CE], MMDT, kind="ExternalInput").ap()
    wco_d = nc.dram_tensor("wco", [N, P, (N - 2) * MO, C], MMDT,
                           kind="ExternalInput").ap()
    wcj_d = nc.dram_tensor("wcj", [N, P, MO, CE], MMDT, kind="ExternalInput").ap()
    wbe_d = nc.dram_tensor("wbe", [N, P, N, C], MMDT, kind="ExternalInput").ap()
    t1_d = nc.dram_tensor("t1c", [P, N, MH], f32, kind="ExternalInput").ap()
    t2_d = nc.dram_tensor("t2c", [P, N, MO], f32, kind="ExternalInput").ap()
    bc_d = nc.dram_tensor("bcc", [C, N], f32, kind="ExternalInput").ap()
    out_d = nc.dram_tensor("out", [N, C, BL], f32, kind="ExternalOutput").ap()

    with tile.TileContext(nc) as tc:
        with (
            tc.tile_pool(name="consts", bufs=1) as consts,
            tc.tile_pool(name="xpool", bufs=2) as xpool,
            tc.tile_pool(name="wpool", bufs=3) as wpool,
            tc.tile_pool(name="hpool", bufs=1) as hpool,
            tc.tile_pool(name="cpool", bufs=1) as cpool,
            tc.tile_pool(name="gpool", bufs=2) as gpool,
            tc.tile_pool(name="pp", bufs=1, space="PSUM") as pp,
        ):
            # PE-critical first loads go ahead of the constants; w1 first
            # (the opening LDWEIGHTS needs only w1m0)
            w1m0 = wpool.tile([P, KF, P], MMDT, tag="w1", name="w1m")
            nc.sync.dma_start(out=w1m0, in_=w1_d[0, :, 0])
            xsb0 = xpool.tile([P, KF, BL], MMDT, tag="x", name="xsb")
            nc.sync.dma_start(out=xsb0[:, 0:2, :], in_=xT_d[0, :, 0:2])

            t1_sb = consts.tile([P, N, MH], f32, tag="t1")
            nc.sync.dma_start(out=t1_sb, in_=t1_d)
            t2_sb = consts.tile([P, N, MO], f32, tag="t2")
            nc.sync.dma_start(out=t2_sb, in_=t2_d)
            bc_sb = consts.tile([C, N], f32, tag="bc")
            nc.sync.dma_start(out=bc_sb, in_=bc_d)
            zeros_sb = consts.tile([P, BL], f32, tag="zeros")
            nc.vector.memset(zeros_sb, 0.0)
            warm_sb = consts.tile([1, 1], f32, tag="warm")
            nc.scalar.activation(out=warm_sb, in_=zeros_sb[0:1, 0:1],
                                 func=AF.Sigmoid, scale=1.0)

            def body(_rep):
                comp_sb = []
                pl_sb = []
                po_sb = []
                se_t = []

                # ---- Stage A: per-node compressors + classifier partials
                # (the SE means ride along in rows 100:118)
                for n in range(N):
                    if _rep == 0 and n == 0 and not hw_loop:
                        xsb = xsb0
                        for kp in range(2, KF, 2):
                            nc.sync.dma_start(out=xsb[:, kp:kp + 2, :],
                                              in_=xT_d[n, :, kp:kp + 2])
                    else:
                        xsb = xpool.tile([P, KF, BL], MMDT, tag="x", name="xsb")
                        # split the 2MB load so L1 can start after the first
                        # pair of f-chunks lands (Tile tracks subtile deps)
                        for kp in range(0, KF, 2):
                            nc.sync.dma_start(out=xsb[:, kp:kp + 2, :],
                                              in_=xT_d[n, :, kp:kp + 2])

                    # L1: h = relu(W1' @ x + t1)
                    hsb = hpool.tile([P, MH, BL], MMDT, tag="h")
                    for m in range(MH):
                        if _rep == 0 and n == 0 and m == 0 and not hw_loop:
                            w1m = w1m0
                        else:
                            w1m = wpool.tile([P, KF, P], MMDT, tag="w1",
                                             name="w1m")
                            nc.sync.dma_start(out=w1m, in_=w1_d[n, :, m])
                        ph = pp.tile([P, BL], f32, tag="h", bufs=2)
                        for k in range(KF):
                            nc.tensor.matmul(ph, w1m[:, k, :], xsb[:, k, :],
                                             start=(k == 0), stop=(k == KF - 1))
                        nc.scalar.activation(out=hsb[:, m, :], in_=ph, func=AF.Relu,
                                             bias=t1_sb[:, n, m:m + 1], scale=1.0)

                    if cut == "l1":
                        ob = gpool.tile([C, BL], f32, tag="osb", bufs=2,
                                        name="ob")
                        nc.vector.tensor_copy(ob, hsb[0:C, 0, :])
                        nc.sync.dma_start(out=out_d[n], in_=ob)
                        continue

                    # L2: comp = relu(W2' @ h + t2)
                    csb = cpool.tile([P, MO, BL], MMDT, tag=f"comp{n}")
                    for o in range(MO):
                        w2m = wpool.tile([P, KH, P], MMDT, tag="w2")
                        nc.sync.dma_start(out=w2m, in_=w2_d[n, :, o])
                        pc = pp.tile([P, BL], f32, tag="c", bufs=3)
                        for k in range(KH):
                            nc.tensor.matmul(pc, w2m[:, k, :], hsb[:, k, :],
                                             start=(k == 0), stop=(k == KH - 1))
                        nc.vector.scalar_tensor_tensor(
                            csb[:, o, :], pc, t2_sb[:, n, o:o + 1], zeros_sb,
                            mybir.AluOpType.add, mybir.AluOpType.max)
                    comp_sb.append(csb)

                    if cut == "l1l2":
                        ob = gpool.tile([C, BL], f32, tag="osb", bufs=2,
                                        name="ob2")
                        nc.vector.tensor_copy(ob, csb[0:C, 0, :])
                        nc.sync.dma_start(out=out_d[n], in_=ob)
                        continue

                    # representative others-classifier group (n0(n), n):
                    # partial for consumer n0(n) + comp-mean SE rows
                    wcj = wpool.tile([P, MO, CE], MMDT, tag="wcj", bufs=2)
                    nc.sync.dma_start(out=wcj, in_=wcj_d[n])
                    ppo = pp.tile([CE, BL], f32, tag="po", bufs=3)
                    for o in range(MO):
                        nc.tensor.matmul(ppo, wcj[:, o, :], csb[:, o, :],
                                         start=(o == 0), stop=(o == MO - 1))
                    po = cpool.tile([CE, BL], f32, tag=f"po{n}")
                    nc.vector.tensor_copy(po, ppo)
                    po_sb.append(po)

                    # ungated local classifier partial + x-mean SE rows
                    wcl = wpool.tile([P, KF, CE], MMDT, tag="wcl")
                    nc.sync.dma_start(out=wcl, in_=wcl_d[n])
                    ppl = pp.tile([CE, BL], f32, tag="po", bufs=3)
                    for k in range(KF):
                        nc.tensor.matmul(ppl, wcl[:, k, :], xsb[:, k, :],
                                         start=(k == 0), stop=(k == KF - 1))
                    pl = cpool.tile([CE, BL], f32, tag=f"pl{n}")
                    nc.vector.tensor_copy(pl, ppl)
                    pl_sb.append(pl)

                    # pair-sum of the SE pre-activation rows (both operands
                    # share base partition 96 — a 32-alignment requirement)
                    t = gpool.tile([32, BL], f32, tag="se", bufs=12, name="t")
                    eng = nc.vector if n % 2 else nc.gpsimd
                    eng.tensor_add(t, pl[96:CE], po[96:CE])
                    se_t.append(t)

                if cut in ("l1", "l1l2"):
                    return

                # ---- Stage B: SE pre-activation tree-sum, then replicate to
                # all four 32-row PE bands (SBUF-SBUF DMA) + one relu, so the
                # gate matmuls can run 3-way row-tiled (concurrent PE bands)
                ei = 0
                while len(se_t) > 2:
                    nxt = []
                    for i in range(0, len(se_t) - 1, 2):
                        s = gpool.tile([32, BL], f32, tag="se", bufs=12, name="s")
                        (nc.vector if ei % 2 else nc.gpsimd).tensor_add(
                            s, se_t[i], se_t[i + 1])
                        ei += 1
                        nxt.append(s)
                    if len(se_t) % 2:
                        nxt.append(se_t[-1])
                    se_t = nxt
                acc = gpool.tile([32, BL], f32, tag="se", bufs=12, name="acc")
                nc.vector.tensor_add(acc, se_t[0], se_t[1])
                # relu first (band 0 gates can start on it immediately), then
                # replicate the bf16 band to the other three PE row bands
                a_sb = consts.tile([P, BL], MMDT, tag="a")
                nc.scalar.activation(out=a_sb[0:32, :], in_=acc, func=AF.Relu,
                                     scale=1.0)
                for i, eng in ((1, nc.vector), (2, nc.gpsimd), (3, nc.vector)):
                    eng.tensor_copy(a_sb[32 * i:32 * i + 32, :], a_sb[0:32, :])

                if cut == "stagea":
                    ob = gpool.tile([C, BL], f32, tag="osb", bufs=2, name="ob3")
                    nc.vector.tensor_copy(ob, pl_sb[0][0:C])
                    nc.vector.tensor_copy(ob[0:32, :], a_sb[0:32, :])
                    nc.sync.dma_start(out=out_d[0], in_=ob)
                    return

                # ---- Stage C: per node, gates first (3-way row-tiled on the
                # PE: tile_position=(32i,0) bands run concurrently; K=32), then
                # the gated classifier pairs whose muls never wait on the gate
                # relay. Node 0 runs three of its classifier pairs ahead of its
                # gates so the PE has work while the SE tree finishes.
                for n in range(N):
                    wbe = wpool.tile([P, N, C], MMDT, tag="wbe")
                    nc.sync.dma_start(out=wbe, in_=wbe_d[n])
                    wco = wpool.tile([P, (N - 2) * MO, C], MMDT, tag="wco",
                                     bufs=2)
                    nc.sync.dma_start(out=wco, in_=wco_d[n])

                    j_skip = (n - 1) % N

                    def emit_pq(ko):
                        pq = pp.tile([CE, BL], f32, tag="po", bufs=3)
                        for o in range(MO):
                            nc.tensor.matmul(pq[0:C, :],
                                             wco[:, ko * MO + o, :],
                                             comp_sb[js[ko]][:, o, :],
                                             start=(o == 0),
                                             stop=(o == MO - 1))
                        return pq

                    js = [int(IDX[n][k]) for k in range(N - 1)
                          if int(IDX[n][k]) != j_skip]
                    pqs = {}
                    if n == 0:
                        for ko in range(3):
                            pqs[ko] = emit_pq(ko)

                    gates = []
                    for m in range(N):
                        i = m % 3
                        pg = pp.tile([P, BL], f32, tag="c", bufs=3)
                        nc.tensor.matmul(pg[0:C, :],
                                         wbe[32 * i:32 * i + 32, m, :],
                                         a_sb[32 * i:32 * i + 32, :],
                                         start=True, stop=True,
                                         tile_position=(32 * i, 0))
                        g = gpool.tile([C, BL], MMDT, tag="gate", bufs=14)
                        nc.scalar.activation(out=g, in_=pg[0:C, :],
                                             func=AF.Sigmoid, scale=1.0)
                        gates.append(g)

                    # gated products, then a binary-tree combine
                    terms = []
                    t0 = gpool.tile([C, BL], f32, tag="tmp", bufs=16)
                    nc.gpsimd.tensor_mul(t0, pl_sb[n][0:C], gates[0])
                    terms.append(t0)
                    ko = 0
                    for k in range(N - 1):
                        j = int(IDX[n][k])
                        tmp = gpool.tile([C, BL], f32, tag="tmp", bufs=16)
                        if j == j_skip:
                            nc.vector.tensor_mul(tmp, po_sb[j][0:C],
                                                 gates[k + 1])
                        else:
                            pq = pqs[ko] if ko in pqs else emit_pq(ko)
                            ko += 1
                            nc.vector.tensor_mul(tmp, pq[0:C, :], gates[k + 1])
                        terms.append(tmp)
                    eng = [nc.vector, nc.gpsimd]
                    ei = 0
                    while len(terms) > 2:
                        nxt = []
                        for i in range(0, len(terms) - 1, 2):
                            s = gpool.tile([C, BL], f32, tag="tmp", bufs=16)
                            eng[ei % 2].tensor_add(s, terms[i], terms[i + 1])
                            ei += 1
                            nxt.append(s)
                        if len(terms) % 2:
                            nxt.append(terms[-1])
                        terms = nxt

                    # final add with the classifier bias folded in:
                    # osb = (terms[0] + bc) + terms[1]
                    osb = gpool.tile([C, BL], f32, tag="osb", bufs=2)
                    nc.vector.scalar_tensor_tensor(
                        osb, terms[0], bc_sb[:, n:n + 1], terms[1],
                        mybir.AluOpType.add, mybir.AluOpType.add)
                    nc.sync.dma_start(out=out_d[n], in_=osb)

            if hw_loop:
                with tc.For_i(0, reps) as _i:
                    for _b in range(bodies_per_iter):
                        body(0)
            else:
                for _rep in range(reps):
                    body(_rep)

    nc.compile()
    return nc


def _host_prep(x, W1, b1, g1, be1, rm1, rv1, W2, b2, g2, be2, rm2, rv2,
               Wa, Wb, Wc, bc):
    f = np.float32
    s1 = (g1 / np.sqrt(rv1 + EPS)).astype(f)               # [N, HID]
    t1 = ((b1 - rm1) * s1 + be1).astype(f)
    W1f = (W1 * s1[:, :, None]).astype(f)                  # [N, HID, F]
    s2 = (g2 / np.sqrt(rv2 + EPS)).astype(f)
    t2 = ((b2 - rm2) * s2 + be2).astype(f)
    W2f = (W2 * s2[:, :, None]).astype(f)                  # [N, FO, HID]

    shared = {}
    # lhsT chunk layouts: [n, p(contraction within chunk), m-chunk, k-chunk, col]
    shared["w1"] = np.ascontiguousarray(
        W1f.reshape(N, MH, P, KF, P).transpose(0, 4, 1, 3, 2))
    shared["w2"] = np.ascontiguousarray(
        W2f.reshape(N, MO, P, KH, P).transpose(0, 4, 1, 3, 2))
    Wc = np.asarray(Wc, dtype=f)
    Wa = np.asarray(Wa, dtype=f)

    # local classifier lhsT with x-mean SE rows appended at cols 100:118
    # (cols 118:128 stay zero so psum rows 96:128 are fully defined):
    #   col 100+3n+r of node n's lhsT = Wa[n, r, 0]/F at every (p, k)
    wcl = np.zeros((N, P, KF, CE), dtype=f)
    wcl[:, :, :, :C] = Wc[:, :, :F].reshape(N, C, KF, P).transpose(0, 3, 2, 1)
    for n in range(N):
        for r in range(RED):
            wcl[n, :, :, C + RED * n + r] = Wa[n, r, 0] / F

    # stage-C others blocks: for consumer n, the (N-2) pairs with j != skip(n)
    # where skip(n) = (n-1) % N (that pair was computed in stage A)
    wco = np.zeros((N, P, (N - 2) * MO, C), dtype=f)
    # representative blocks: for source j, consumer n0 = (j+1) % N,
    # plus comp-mean SE rows: col 100+3m+r = Wa[m, r, 1+kpos]/FO where
    # IDX[m][kpos] == j (zero for m == j)
    wcj = np.zeros((N, P, MO, CE), dtype=f)
    WcO = Wc[:, :, F:].reshape(N, C, N - 1, MO, P)   # [n, c, kpos, o, p]
    for n in range(N):
        j_skip = (n - 1) % N
        ko = 0
        for kpos in range(N - 1):
            j = int(IDX[n][kpos])
            if j == j_skip:
                continue
            wco[n, :, ko * MO:(ko + 1) * MO, :] = \
                WcO[n, :, kpos].transpose(2, 1, 0)
            ko += 1
    for j in range(N):
        n0 = N0[j]
        kpos = list(IDX[n0]).index(j)
        wcj[j, :, :, :C] = WcO[n0, :, kpos].transpose(2, 1, 0)
        for m in range(N):
            if m == j:
                continue
            kp = list(IDX[m]).index(j)
            for r in range(RED):
                wcj[j, :, :, C + RED * m + r] = Wa[m, r, 1 + kp] / FO

    shared["wcl"] = wcl
    shared["wco"] = wco
    shared["wcj"] = wcj

    # SE stage-2, pre-broadcast, 32-row rhs layout: the relu'd SE pre-acts sit
    # at rows 4:22 of each 32-row band of the a-tile (psum rows 96:128 with SE
    # at 100:118), so wbe row 4 + n*3 + r carries Wb[n, m, r]; the band is
    # replicated 4x so gate matmuls can row-tile to any PE band.
    Wb = np.asarray(Wb, dtype=f)
    wbe = np.zeros((N, 32, N, C), dtype=f)
    for n in range(N):
        for m in range(N):
            for r in range(RED):
                wbe[n, 4 + n * RED + r, m, :] = Wb[n, m, r]
    shared["wbe"] = np.ascontiguousarray(np.tile(wbe, (1, 4, 1, 1)))

    shared["t1c"] = np.ascontiguousarray(t1.reshape(N, MH, P).transpose(2, 0, 1))
    shared["t2c"] = np.ascontiguousarray(t2.reshape(N, MO, P).transpose(2, 0, 1))
    shared["bcc"] = np.ascontiguousarray(np.asarray(bc, dtype=f).T)

    import ml_dtypes
    mmnp = ml_dtypes.bfloat16
    for k in ("w1", "w2", "wcl", "wco", "wcj", "wbe"):
        shared[k] = shared[k].astype(mmnp)

    x = np.asarray(x, dtype=f)
    in_maps = []
    for i in range(NCORES):
        xi = x[i * BL:(i + 1) * BL]                        # [BL, N, F]
        xt = np.ascontiguousarray(
            xi.transpose(1, 2, 0).reshape(N, KF, P, BL).transpose(0, 2, 1, 3))
        xt = xt.astype(mmnp)
        m = dict(shared)
        m["xT"] = xt
        in_maps.append(m)
    return in_maps


def kernel(**inputs):
    global LAST_EXEC_TIME_NS
    if "nc" not in _BUILT:
        _BUILT["nc"] = _build_nc()
    nc = _BUILT["nc"]

    inputs = {k: np.asarray(v) for k, v in inputs.items()}
    in_maps = _host_prep(**inputs)
    res = run_bass_kernel_spmd(nc, in_maps, core_ids=list(range(NCORES)))
    LAST_EXEC_TIME_NS = res.exec_time_ns

    out = np.empty((B, N, C), dtype=np.float32)
    for i in range(NCORES):
        out[i * BL:(i + 1) * BL] = res.results[i]["out"].transpose(2, 0, 1)
    return out


# revision 34
# speedup vs baseline: 1.0802x; 1.0802x over previous
"""Trainium2 Bass kernel for the per-node compressor + SE-gate + classifier model.

Strategy: data-parallel over batch B across 8 NeuronCores (512 rows each).
On-chip layout is feature-major [feature, batch]; BatchNorm is folded into the
linear weights on the host. The SE pooling means ride for free in spare lhsT
columns of the classifier matmuls: the local classifier emits 118 rows
(100 classes + 18 x-mean SE rows), and one "representative" others-classifier
group per source node emits 118 rows (100 classes + 18 comp-mean SE rows),
so no dedicated mean matmuls are needed. SE gates are produced as
partition-broadcast [100, 512] tiles from tiny matmuls; the classifier is
computed as per-block ungated partials which are gated post-matmul on
VectorE/Pool. Matmul operands are bf16; accumulation and gating stay fp32.
"""

import numpy as np

import concourse.bass as bass
import concourse.tile as tile
from concourse import bacc, mybir
from concourse.bass_utils import run_bass_kernel_spmd

# Problem shapes (hardcoded per harness contract)
B, N, F, FO, C = 4096, 6, 1024, 512, 100
HID = (F + FO) // 2          # 768
RED = N // 2                 # 3
EPS = 1e-5
IDX = np.array([[j for j in range(N) if j != i] for i in range(N)])

NCORES = 8
BL = B // NCORES             # 512 batch rows per core
P = 128
KF = F // P                  # 8 f-chunks
MH = HID // P                # 6 h-chunks
KH = HID // P                # 6
MO = FO // P                 # 4 o-chunks
CE = P                       # classifier psum rows: 100 classes + 18 SE-mean
                             # rows at 100:118 + zero padding to 128 (so the
                             # 32-aligned [96:128] slice is fully defined)

f32 = mybir.dt.float32
bf16 = mybir.dt.bfloat16
MMDT = bf16
AF = mybir.ActivationFunctionType

# representative consumer for source node j: its (n0, j) classifier group is
# computed in stage A (as soon as comp[j] exists) and carries the comp-mean
# SE rows; node n0 then skips that pair in stage C.
N0 = [(j + 1) % N for j in range(N)]

LAST_EXEC_TIME_NS = None

_BUILT = {}


def _build_nc(reps=1, hw_loop=False, cut=None, bodies_per_iter=1):
    # cut: None=full body, "l1"=L1 only, "l1l2"=compressors only,
    # "stagea"=through SE pre-acts (perf bisection only; kernel() uses full)
    nc = bacc.Bacc("TRN2", target_bir_lowering=False, debug=False,
                   num_devices=NCORES)

    xT_d = nc.dram_tensor("xT", [N, P, KF, BL], MMDT, kind="ExternalInput").ap()
    w1_d = nc.dram_tensor("w1", [N, P, MH, KF, P], MMDT, kind="ExternalInput").ap()
    w2_d = nc.dram_tensor("w2", [N, P, MO, KH, P], MMDT, kind="ExternalInput").ap()
    wcl_d = nc.dram_tensor("wcl", [N, P, KF, CE], MMDT, kind="ExternalInput").ap()
    wco_d = nc.dram_tensor("wco", [N, P, (N - 2) * MO, C], MMDT,
                           kind="ExternalInput").ap()
    wcj_d = nc.dram_tensor("wcj", [N, P, MO, CE], MMDT, kind="ExternalInput").ap()
    wbe_d = nc.dram_tensor("wbe", [N, P, N, C], MMDT, kind="ExternalInput").ap()
    t1_d = nc.dram_tensor("t1c", [P, N, MH], f32, kind="ExternalInput").ap()
    t2_d = nc.dram_tensor("t2c", [P, N, MO], f32, kind="ExternalInput").ap()
    bc_d = nc.dram_tensor("bcc", [C, N], f32, kind="ExternalInput").ap()
    out_d = nc.dram_tensor("out", [N, C, BL], f32, kind="ExternalOutput").ap()

    with tile.TileContext(nc) as tc:
        with (
            tc.tile_pool(name="consts", bufs=1) as consts,
            tc.tile_pool(name="xpool", bufs=3) as xpool,
            tc.tile_pool(name="wpool", bufs=3) as wpool,
            tc.tile_pool(name="hpool", bufs=1) as hpool,
            tc.tile_pool(name="cpool", bufs=1) as cpool,
            tc.tile_pool(name="gpool", bufs=2) as gpool,
            tc.tile_pool(name="pp", bufs=1, space="PSUM") as pp,
        ):
            # PE-critical first loads go ahead of the constants; w1 first
            # (the opening LDWEIGHTS needs only w1m0)
            w1m0 = wpool.tile([P, KF, P], MMDT, tag="w1", name="w1m", bufs=4)
            nc.sync.dma_start(out=w1m0, in_=w1_d[0, :, 0])
            xsb0 = xpool.tile([P, KF, BL], MMDT, tag="x", name="xsb")
            nc.sync.dma_start(out=xsb0[:, 0:2, :], in_=xT_d[0, :, 0:2])

            t1_sb = consts.tile([P, N, MH], f32, tag="t1")
            nc.sync.dma_start(out=t1_sb, in_=t1_d)
            t2_sb = consts.tile([P, N, MO], f32, tag="t2")
            nc.sync.dma_start(out=t2_sb, in_=t2_d)
            bc_sb = consts.tile([C, N], f32, tag="bc")
            nc.sync.dma_start(out=bc_sb, in_=bc_d)
            zeros_sb = consts.tile([P, BL], f32, tag="zeros")
            nc.vector.memset(zeros_sb, 0.0)
            warm_sb = consts.tile([1, 1], f32, tag="warm")
            nc.scalar.activation(out=warm_sb, in_=zeros_sb[0:1, 0:1],
                                 func=AF.Sigmoid, scale=1.0)

            def body(_rep):
                comp_sb = []
                pl_sb = []
                po_sb = []
                se_t = []

                # ---- Stage A: per-node compressors + classifier partials
                # (the SE means ride along in rows 100:118)
                for n in range(N):
                    if _rep == 0 and n == 0 and not hw_loop:
                        xsb = xsb0
                        for kp in range(2, KF, 2):
                            nc.sync.dma_start(out=xsb[:, kp:kp + 2, :],
                                              in_=xT_d[n, :, kp:kp + 2])
                    else:
                        xsb = xpool.tile([P, KF, BL], MMDT, tag="x", name="xsb")
                        # split the 2MB load so L1 can start after the first
                        # pair of f-chunks lands (Tile tracks subtile deps)
                        for kp in range(0, KF, 2):
                            nc.sync.dma_start(out=xsb[:, kp:kp + 2, :],
                                              in_=xT_d[n, :, kp:kp + 2])

                    # L1: h = relu(W1' @ x + t1)
                    hsb = hpool.tile([P, MH, BL], MMDT, tag="h")
                    for m in range(MH):
                        if _rep == 0 and n == 0 and m == 0 and not hw_loop:
                            w1m = w1m0
                        else:
                            w1m = wpool.tile([P, KF, P], MMDT, tag="w1",
                                             name="w1m", bufs=4)
                            nc.sync.dma_start(out=w1m, in_=w1_d[n, :, m])
                        ph = pp.tile([P, BL], f32, tag="h", bufs=2)
                        for k in range(KF):
                            nc.tensor.matmul(ph, w1m[:, k, :], xsb[:, k, :],
                                             start=(k == 0), stop=(k == KF - 1))
                        nc.scalar.activation(out=hsb[:, m, :], in_=ph, func=AF.Relu,
                                             bias=t1_sb[:, n, m:m + 1], scale=1.0)

                    if cut == "l1":
                        ob = gpool.tile([C, BL], f32, tag="osb", bufs=2,
                                        name="ob")
                        nc.vector.tensor_copy(ob, hsb[0:C, 0, :])
                        nc.sync.dma_start(out=out_d[n], in_=ob)
                        continue

                    # L2: comp = relu(W2' @ h + t2)
                    csb = cpool.tile([P, MO, BL], MMDT, tag=f"comp{n}")
                    for o in range(MO):
                        w2m = wpool.tile([P, KH, P], MMDT, tag="w2")
                        nc.sync.dma_start(out=w2m, in_=w2_d[n, :, o])
                        pc = pp.tile([P, BL], f32, tag="c", bufs=3)
                        for k in range(KH):
                            nc.tensor.matmul(pc, w2m[:, k, :], hsb[:, k, :],
                                             start=(k == 0), stop=(k == KH - 1))
                        nc.vector.scalar_tensor_tensor(
                            csb[:, o, :], pc, t2_sb[:, n, o:o + 1], zeros_sb,
                            mybir.AluOpType.add, mybir.AluOpType.max)
                    comp_sb.append(csb)

                    if cut == "l1l2":
                        ob = gpool.tile([C, BL], f32, tag="osb", bufs=2,
                                        name="ob2")
                        nc.vector.tensor_copy(ob, csb[0:C, 0, :])
                        nc.sync.dma_start(out=out_d[n], in_=ob)
                        continue

                    # representative others-classifier group (n0(n), n):
                    # partial for consumer n0(n) + comp-mean SE rows
                    wcj = wpool.tile([P, MO, CE], MMDT, tag="wcj", bufs=2)
                    nc.sync.dma_start(out=wcj, in_=wcj_d[n])
                    ppo = pp.tile([CE, BL], f32, tag="po", bufs=3)
                    for o in range(MO):
                        nc.tensor.matmul(ppo, wcj[:, o, :], csb[:, o, :],
                                         start=(o == 0), stop=(o == MO - 1))
                    po = cpool.tile([CE, BL], f32, tag=f"po{n}")
                    nc.vector.tensor_copy(po, ppo)
                    po_sb.append(po)

                    # ungated local classifier partial + x-mean SE rows
                    wcl = wpool.tile([P, KF, CE], MMDT, tag="wcl")
                    nc.sync.dma_start(out=wcl, in_=wcl_d[n])
                    ppl = pp.tile([CE, BL], f32, tag="po", bufs=3)
                    for k in range(KF):
                        nc.tensor.matmul(ppl, wcl[:, k, :], xsb[:, k, :],
                                         start=(k == 0), stop=(k == KF - 1))
                    pl = cpool.tile([CE, BL], f32, tag=f"pl{n}")
                    nc.vector.tensor_copy(pl, ppl)
                    pl_sb.append(pl)

                    # pair-sum of the SE pre-activation rows (both operands
                    # share base partition 96 — a 32-alignment requirement)
                    t = gpool.tile([32, BL], f32, tag="se", bufs=12, name="t")
                    eng = nc.vector if n % 2 else nc.gpsimd
                    eng.tensor_add(t, pl[96:CE], po[96:CE])
                    se_t.append(t)

                if cut in ("l1", "l1l2"):
                    return

                # ---- Stage B: SE pre-activation tree-sum, then replicate to
                # all four 32-row PE bands (SBUF-SBUF DMA) + one relu, so the
                # gate matmuls can run 3-way row-tiled (concurrent PE bands)
                ei = 0
                while len(se_t) > 2:
                    nxt = []
                    for i in range(0, len(se_t) - 1, 2):
                        s = gpool.tile([32, BL], f32, tag="se", bufs=12, name="s")
                        (nc.vector if ei % 2 else nc.gpsimd).tensor_add(
                            s, se_t[i], se_t[i + 1])
                        ei += 1
                        nxt.append(s)
                    if len(se_t) % 2:
                        nxt.append(se_t[-1])
                    se_t = nxt
                acc4 = consts.tile([P, BL], f32, tag="acc4")
                nc.vector.tensor_add(acc4[0:32, :], se_t[0], se_t[1])
                for i in range(1, 4):
                    nc.sync.dma_start(out=acc4[32 * i:32 * i + 32, :],
                                      in_=acc4[0:32, :])
                a_sb = consts.tile([P, BL], MMDT, tag="a")
                nc.scalar.activation(out=a_sb, in_=acc4, func=AF.Relu,
                                     scale=1.0)

                if cut == "stagea":
                    ob = gpool.tile([C, BL], f32, tag="osb", bufs=2, name="ob3")
                    nc.vector.tensor_copy(ob, pl_sb[0][0:C])
                    nc.vector.tensor_copy(ob[0:32, :], a_sb[0:32, :])
                    nc.sync.dma_start(out=out_d[0], in_=ob)
                    return

                # ---- Stage C: per node, gates first (3-way row-tiled on the
                # PE: tile_position=(32i,0) bands run concurrently; K=32), then
                # the gated classifier pairs whose muls never wait on the gate
                # relay. Node 0 runs three of its classifier pairs ahead of its
                # gates so the PE has work while the SE tree finishes.
                for n in range(N):
                    wbe = wpool.tile([P, N, C], MMDT, tag="wbe")
                    nc.sync.dma_start(out=wbe, in_=wbe_d[n])
                    wco = wpool.tile([P, (N - 2) * MO, C], MMDT, tag="wco",
                                     bufs=2)
                    nc.sync.dma_start(out=wco, in_=wco_d[n])

                    j_skip = (n - 1) % N

                    def emit_pq(ko):
                        pq = pp.tile([CE, BL], f32, tag="po", bufs=3)
                        for o in range(MO):
                            nc.tensor.matmul(pq[0:C, :],
                                             wco[:, ko * MO + o, :],
                                             comp_sb[js[ko]][:, o, :],
                                             start=(o == 0),
                                             stop=(o == MO - 1))
                        return pq

                    js = [int(IDX[n][k]) for k in range(N - 1)
                          if int(IDX[n][k]) != j_skip]
                    pqs = {}
                    if n == 0:
                        for ko in range(3):
                            pqs[ko] = emit_pq(ko)

                    gates = []
                    for m in range(N):
                        i = m % 3
                        pg = pp.tile([P, BL], f32, tag="c", bufs=3)
                        nc.tensor.matmul(pg[0:C, :],
                                         wbe[32 * i:32 * i + 32, m, :],
                                         a_sb[32 * i:32 * i + 32, :],
                                         start=True, stop=True,
                                         tile_position=(32 * i, 0))
                        g = gpool.tile([C, BL], MMDT, tag="gate", bufs=14)
                        nc.scalar.activation(out=g, in_=pg[0:C, :],
                                             func=AF.Sigmoid, scale=1.0)
                        gates.append(g)

                    # gated products, then a binary-tree combine
                    terms = []
                    t0 = gpool.tile([C, BL], f32, tag="tmp", bufs=16)
                    nc.gpsimd.tensor_mul(t0, pl_sb[n][0:C], gates[0])
                    terms.append(t0)
                    ko = 0
                    for k in range(N - 1):
                        j = int(IDX[n][k])
                        tmp = gpool.tile([C, BL], f32, tag="tmp", bufs=16)
                        if j == j_skip:
                            nc.vector.tensor_mul(tmp, po_sb[j][0:C],
                                                 gates[k + 1])
                        else:
                            pq = pqs[ko] if ko in pqs else emit_pq(ko)
                            ko += 1
                            nc.vector.tensor_mul(tmp, pq[0:C, :], gates[k + 1])
                        terms.append(tmp)
                    eng = [nc.vector, nc.gpsimd]
                    ei = 0
                    while len(terms) > 2:
                        nxt = []
                        for i in range(0, len(terms) - 1, 2):
                            s = gpool.tile([C, BL], f32, tag="tmp", bufs=16)
                            eng[ei % 2].tensor_add(s, terms[i], terms[i + 1])
                            ei += 1
                            nxt.append(s)
                        if len(terms) % 2:
                            nxt.append(terms[-1])
                        terms = nxt

                    # final add with the classifier bias folded in:
                    # osb = (terms[0] + bc) + terms[1]
                    osb = gpool.tile([C, BL], f32, tag="osb", bufs=2)
                    nc.vector.scalar_tensor_tensor(
                        osb, terms[0], bc_sb[:, n:n + 1], terms[1],
                        mybir.AluOpType.add, mybir.AluOpType.add)
                    nc.sync.dma_start(out=out_d[n], in_=osb)

            if hw_loop:
                with tc.For_i(0, reps) as _i:
                    for _b in range(bodies_per_iter):
                        body(0)
            else:
                for _rep in range(reps):
                    body(_rep)

    nc.compile()
    return nc


def _host_prep(x, W1, b1, g1, be1, rm1, rv1, W2, b2, g2, be2, rm2, rv2,
               Wa, Wb, Wc, bc):
    f = np.float32
    s1 = (g1 / np.sqrt(rv1 + EPS)).astype(f)               # [N, HID]
    t1 = ((b1 - rm1) * s1 + be1).astype(f)
    W1f = (W1 * s1[:, :, None]).astype(f)                  # [N, HID, F]
    s2 = (g2 / np.sqrt(rv2 + EPS)).astype(f)
    t2 = ((b2 - rm2) * s2 + be2).astype(f)
    W2f = (W2 * s2[:, :, None]).astype(f)                  # [N, FO, HID]

    shared = {}
    # lhsT chunk layouts: [n, p(contraction within chunk), m-chunk, k-chunk, col]
    shared["w1"] = np.ascontiguousarray(
        W1f.reshape(N, MH, P, KF, P).transpose(0, 4, 1, 3, 2))
    shared["w2"] = np.ascontiguousarray(
        W2f.reshape(N, MO, P, KH, P).transpose(0, 4, 1, 3, 2))
    Wc = np.asarray(Wc, dtype=f)
    Wa = np.asarray(Wa, dtype=f)

    # local classifier lhsT with x-mean SE rows appended at cols 100:118
    # (cols 118:128 stay zero so psum rows 96:128 are fully defined):
    #   col 100+3n+r of node n's lhsT = Wa[n, r, 0]/F at every (p, k)
    wcl = np.zeros((N, P, KF, CE), dtype=f)
    wcl[:, :, :, :C] = Wc[:, :, :F].reshape(N, C, KF, P).transpose(0, 3, 2, 1)
    for n in range(N):
        for r in range(RED):
            wcl[n, :, :, C + RED * n + r] = Wa[n, r, 0] / F

    # stage-C others blocks: for consumer n, the (N-2) pairs with j != skip(n)
    # where skip(n) = (n-1) % N (that pair was computed in stage A)
    wco = np.zeros((N, P, (N - 2) * MO, C), dtype=f)
    # representative blocks: for source j, consumer n0 = (j+1) % N,
    # plus comp-mean SE rows: col 100+3m+r = Wa[m, r, 1+kpos]/FO where
    # IDX[m][kpos] == j (zero for m == j)
    wcj = np.zeros((N, P, MO, CE), dtype=f)
    WcO = Wc[:, :, F:].reshape(N, C, N - 1, MO, P)   # [n, c, kpos, o, p]
    for n in range(N):
        j_skip = (n - 1) % N
        ko = 0
        for kpos in range(N - 1):
            j = int(IDX[n][kpos])
            if j == j_skip:
                continue
            wco[n, :, ko * MO:(ko + 1) * MO, :] = \
                WcO[n, :, kpos].transpose(2, 1, 0)
            ko += 1
    for j in range(N):
        n0 = N0[j]
        kpos = list(IDX[n0]).index(j)
        wcj[j, :, :, :C] = WcO[n0, :, kpos].transpose(2, 1, 0)
        for m in range(N):
            if m == j:
                continue
            kp = list(IDX[m]).index(j)
            for r in range(RED):
                wcj[j, :, :, C + RED * m + r] = Wa[m, r, 1 + kp] / FO

    shared["wcl"] = wcl
    shared["wco"] = wco
    shared["wcj"] = wcj

    # SE stage-2, pre-broadcast, 32-row rhs layout: the relu'd SE pre-acts sit
    # at rows 4:22 of each 32-row band of the a-tile (psum rows 96:128 with SE
    # at 100:118), so wbe row 4 + n*3 + r carries Wb[n, m, r]; the band is
    # replicated 4x so gate matmuls can row-tile to any PE band.
    Wb = np.asarray(Wb, dtype=f)
    wbe = np.zeros((N, 32, N, C), dtype=f)
    for n in range(N):
        for m in range(N):
            for r in range(RED):
                wbe[n, 4 + n * RED + r, m, :] = Wb[n, m, r]
    shared["wbe"] = np.ascontiguousarray(np.tile(wbe, (1, 4, 1, 1)))

    shared["t1c"] = np.ascontiguousarray(t1.reshape(N, MH, P).transpose(2, 0, 1))
    shared["t2c"] = np.ascontiguousarray(t2.reshape(N, MO, P).transpose(2, 0, 1))
    shared["bcc"] = np.ascontiguousarray(np.asarray(bc, dtype=f).T)

    import ml_dtypes
    mmnp = ml_dtypes.bfloat16
    for k in ("w1", "w2", "wcl", "wco", "wcj", "wbe"):
        shared[k] = shared[k].astype(mmnp)

    x = np.asarray(x, dtype=f)
    in_maps = []
    for i in range(NCORES):
        xi = x[i * BL:(i + 1) * BL]                        # [BL, N, F]
        xt = np.ascontiguousarray(
            xi.transpose(1, 2, 0).reshape(N, KF, P, BL).transpose(0, 2, 1, 3))
        xt = xt.astype(mmnp)
        m = dict(shared)
        m["xT"] = xt
        in_maps.append(m)
    return in_maps


def kernel(**inputs):
    global LAST_EXEC_TIME_NS
    if "nc" not in _BUILT:
        _BUILT["nc"] = _build_nc()
    nc = _BUILT["nc"]

    inputs = {k: np.asarray(v) for k, v in inputs.items()}
    in_maps = _host_prep(**inputs)
    res = run_bass_kernel_spmd(nc, in_maps, core_ids=list(range(NCORES)))
    LAST_EXEC_TIME_NS = res.exec_time_ns

    out = np.empty((B, N, C), dtype=np.float32)
    for i in range(NCORES):
        out[i * BL:(i + 1) * BL] = res.results[i]["out"].transpose(2, 0, 1)
    return out
